# revision 25
# baseline (speedup 1.0000x reference)
"""Trainium2 Bass kernel for the Cheirality loss layer.

Math (per batch b, pixel (y, x); g = grad_dirs, n = normal_flow):
    AV0 = V2*x - V0                    AV1 = V2*y - V1
    BW0 = O0*x*y - O1*(x^2+1) + O2*y   BW1 = O0*(y^2+1) - O1*x*y - O2*x
    rho = (g0*AV0 + g1*AV1) * (n0 + n1 - g0*BW0 - g1*BW1)
    out = mean(gelu(-rho))             (exact erf-based gelu)

Device dataflow (v6):
    DVE (all bf16 2x tensor_tensor):
        P1=g0*x  P2=g1*y  P3=g0*y  P4=g1*x  u=P1+P2  XU=x*u  YU=y*u
        rho = dot1m * NEG
    TensorEngine (fp16 diagonal lhsT, fp32 PSUM accumulation):
        NEG  = O0*YU - O1*XU + O2*P3 - O2*P4 - O1*g0 + O0*g1 - n0 - n1
             ( = g.BW - n0 - n1 = -r2 )
        dot1m= -V2*u + V0*g0 + V1*g1  ( = -dot1 )
        so dot1m*NEG = dot1*r2 = rho
    ACT: PSUM->bf16 copies of NEG/dot1m; gelu(-rho) with accumulate.
    GPSIMD: idle (its SBUF traffic slows concurrent DVE ops ~3.4x).

Sharding: pure data parallel, 2 batches per core, batch pair interleaved on
partition halves (batch b -> partitions [64b, 64b+64)). All pose-dependent
coefficients enter via host-built fp16 diagonal matrices whose diagonal is
per-partition (so one pass covers both batches). grad_dirs/normal_flow are
host-cast to bf16 (the device pipeline computes in bf16 anyway) and
host-interleaved to [128, 2, 4800] so each chunk is one 128-partition DMA
per tensor. x/y grids stream as fp16 (exact for ints < 2048).
Reduction: ACT accum_out -> [128, NCHUNK] partials, host sums in float64.
"""

import numpy as np
import ml_dtypes

import concourse.bacc as bacc
import concourse.bass as bass
import concourse.tile as tile
from concourse import mybir
from concourse.bass_utils import run_bass_kernel_spmd

# Problem geometry (hardcoded per the task contract).
B, H, W = 16, 480, 640
NPIX = H * W            # 307200
NCORES = 8
BPC = B // NCORES       # 2 batches per core
PHALF = 64              # partitions per batch
FTOT = NPIX // PHALF    # 4800 free elems per partition
# tapered chunks: small first chunk starts compute sooner, small last chunk
# shortens the final dependency chain
CHUNKS = [480, 960, 960, 960, 960, 480]
NCHUNK = len(CHUNKS)
FCMAX = max(CHUNKS)
MMF = 512               # max matmul free dim (one PSUM bank)

F32 = mybir.dt.float32
F16 = mybir.dt.float16
BF16 = mybir.dt.bfloat16
AF = mybir.ActivationFunctionType

# diag slots in the `diags` input
D_W0, D_W1N, D_W2, D_W2N, D_NI, D_V2N, D_V0, D_V1 = range(8)
NDIAG = 8


def _build_kernel(tc, gd, nf, xyg, diags, out):
    nc = tc.nc
    gd_t = gd.ap()
    nf_t = nf.ap()
    xy_t = xyg.ap()

    with (
        tc.tile_pool(name="singles", bufs=1) as singles,
        tc.tile_pool(name="ins", bufs=6) as ins,
        tc.tile_pool(name="mids", bufs=2) as mids,
        tc.tile_pool(name="psum", bufs=2, space="PSUM") as psp,
    ):
        dg = singles.tile([128, NDIAG, 128], F16, name="dg")
        acc = singles.tile([128, NCHUNK], F32, name="acc")

        f0s = [sum(CHUNKS[:i]) for i in range(NCHUNK)]
        for ci in range(NCHUNK):
            FC = CHUNKS[ci]
            sl = slice(f0s[ci], f0s[ci] + FC)
            gdt = ins.tile([128, 2, FCMAX], BF16, tag="gdt", name=f"gdt_{ci}")[:, :, :FC]
            nft = ins.tile([128, 2, FCMAX], BF16, tag="nft", name=f"nft_{ci}")[:, :, :FC]
            xyt = ins.tile([128, 2, FCMAX], F16, tag="xy", name=f"xy_{ci}")[:, :, :FC]
            nc.sync.dma_start(out=gdt, in_=gd_t[:, :, sl])
            nc.sync.dma_start(out=xyt, in_=xy_t[:, :, sl])
            nc.sync.dma_start(out=nft, in_=nf_t[:, :, sl])
            if ci == 0:
                # diags aren't needed until the first coefficient matmuls;
                # issue their DMA after chunk 0's data
                nc.sync.dma_start(out=dg, in_=diags.ap().rearrange("d k m -> k d m"))
                DG = [dg[:, i, :] for i in range(NDIAG)]
            g0 = gdt[:, 0]
            g1 = gdt[:, 1]
            n0 = nft[:, 0]
            n1 = nft[:, 1]
            xt = xyt[:, 0]
            yt = xyt[:, 1]

            def mtile(tag, dt=BF16):
                return mids.tile([128, FCMAX], dt, tag=tag, name=f"{tag}_{ci}")[:, :FC]

            # DVE products (all 16-bit, 2x mode)
            P1 = mtile("P1")
            nc.vector.tensor_mul(out=P1, in0=g0, in1=xt)
            P2 = mtile("P2")
            nc.vector.tensor_mul(out=P2, in0=g1, in1=yt)
            P3 = mtile("P3")
            nc.vector.tensor_mul(out=P3, in0=g0, in1=yt)
            P4 = mtile("P4")
            nc.vector.tensor_mul(out=P4, in0=g1, in1=xt)
            u = mtile("u")
            nc.vector.tensor_add(out=u, in0=P1, in1=P2)
            XU = mtile("XU")
            nc.vector.tensor_mul(out=XU, in0=xt, in1=u)
            YU = mtile("YU")
            nc.vector.tensor_mul(out=YU, in0=yt, in1=u)

            # PE: NEG = g.BW - n0 - n1 (ordered to reuse stationary weights)
            neg_ps = psp.tile([128, FCMAX], F32, tag="neg", name=f"neg_{ci}")[:, :FC]
            neg_terms = [
                (D_NI, n0), (D_NI, n1),
                (D_W0, YU), (D_W0, g1),
                (D_W1N, XU), (D_W1N, g0),
                (D_W2, P3), (D_W2N, P4),
            ]
            # PE: dot1m = -V2*u + V0*g0 + V1*g1
            d1_ps = psp.tile([128, FCMAX], F32, tag="d1", name=f"d1_{ci}")[:, :FC]
            d1_terms = [(D_V2N, u), (D_V0, g0), (D_V1, g1)]
            for ps, terms in ((neg_ps, neg_terms), (d1_ps, d1_terms)):
                for f0 in range(0, FC, MMF):
                    fs = slice(f0, min(f0 + MMF, FC))
                    for i, (di, rhs) in enumerate(terms):
                        nc.tensor.matmul(
                            ps[:, fs], DG[di], rhs[:, fs],
                            start=(i == 0), stop=(i == len(terms) - 1),
                        )

            negb = mtile("negb")
            nc.scalar.activation(out=negb, in_=neg_ps, func=AF.Copy)
            d1b = mtile("d1b")
            nc.scalar.activation(out=d1b, in_=d1_ps, func=AF.Copy)

            rho = mtile("rho")
            nc.vector.tensor_mul(out=rho, in0=d1b, in1=negb)
            gl = mtile("gl")
            nc.scalar.activation(
                out=gl, in_=rho, func=AF.Gelu, bias=0.0, scale=-1.0,
                accum_out=acc[:, ci : ci + 1],
            )

        nc.sync.dma_start(out=out.ap(), in_=acc)


def build_bass():
    nc = bacc.Bacc("TRN2", target_bir_lowering=False, debug=False)
    gd = nc.dram_tensor("gd", [128, 2, FTOT], BF16, kind="ExternalInput")
    nf = nc.dram_tensor("nf", [128, 2, FTOT], BF16, kind="ExternalInput")
    xyg = nc.dram_tensor("xyg", [128, 2, FTOT], F16, kind="ExternalInput")
    diags = nc.dram_tensor("diags", [NDIAG, 128, 128], F16, kind="ExternalInput")
    out = nc.dram_tensor("acc_out", [128, NCHUNK], F32, kind="ExternalOutput")
    with tile.TileContext(nc) as tc:
        _build_kernel(tc, gd, nf, xyg, diags, out)
    nc.compile()
    return nc


def make_in_maps(pose, grad_dirs, normal_flow):
    pose = np.asarray(pose, np.float32)
    gd = np.ascontiguousarray(np.asarray(grad_dirs, np.float32)).reshape(B, 2, NPIX)
    nf = np.ascontiguousarray(np.asarray(normal_flow, np.float32)).reshape(B, 2, NPIX)

    flat = np.arange(NPIX, dtype=np.int64).reshape(PHALF, FTOT)
    xy_half = np.stack([(flat % W), (flat // W)], axis=1)  # [64, 2, FTOT]
    xyg = np.ascontiguousarray(
        np.concatenate([xy_half, xy_half], axis=0).astype(np.float16)
    )  # [128, 2, FTOT]

    def interleave(a):
        # [BPC, 2, NPIX] -> [128, 2, FTOT] bf16; batch b -> partitions [64b, ...)
        return np.ascontiguousarray(
            a.reshape(BPC, 2, PHALF, FTOT)
            .transpose(0, 2, 1, 3)
            .reshape(128, 2, FTOT)
            .astype(ml_dtypes.bfloat16)
        )

    in_maps = []
    for core in range(NCORES):
        b0 = core * BPC
        # per-partition coefficient vectors (batch = partition // 64)
        coef = np.zeros((NDIAG, 128), np.float32)
        for h in range(BPC):
            V = pose[b0 + h, :3]
            O = pose[b0 + h, 3:]
            rows = slice(h * PHALF, (h + 1) * PHALF)
            coef[D_W0, rows] = O[0]
            coef[D_W1N, rows] = -O[1]
            coef[D_W2, rows] = O[2]
            coef[D_W2N, rows] = -O[2]
            coef[D_NI, rows] = -1.0
            coef[D_V2N, rows] = -V[2]
            coef[D_V0, rows] = V[0]
            coef[D_V1, rows] = V[1]
        diags = np.zeros((NDIAG, 128, 128), np.float16)
        for i in range(NDIAG):
            np.fill_diagonal(diags[i], coef[i].astype(np.float16))
        in_maps.append(
            {
                "gd": interleave(gd[b0 : b0 + BPC]),
                "nf": interleave(nf[b0 : b0 + BPC]),
                "xyg": xyg,
                "diags": diags,
            }
        )
    return in_maps


_NC_CACHE = None


def _get_nc():
    global _NC_CACHE
    if _NC_CACHE is None:
        _NC_CACHE = build_bass()
    return _NC_CACHE


def kernel(pose, grad_dirs, normal_flow):
    nc = _get_nc()
    in_maps = make_in_maps(pose, grad_dirs, normal_flow)
    res = run_bass_kernel_spmd(nc, in_maps, core_ids=list(range(NCORES)))
    total = 0.0
    for r in res.results:
        total += r["acc_out"].astype(np.float64).sum()
    return np.float32(total / (B * H * W))


# revision 26
# speedup vs baseline: 1.0027x; 1.0027x over previous
"""Trainium2 Bass kernel for the Cheirality loss layer.

Math (per batch b, pixel (y, x); g = grad_dirs, n = normal_flow):
    AV0 = V2*x - V0                    AV1 = V2*y - V1
    BW0 = O0*x*y - O1*(x^2+1) + O2*y   BW1 = O0*(y^2+1) - O1*x*y - O2*x
    rho = (g0*AV0 + g1*AV1) * (n0 + n1 - g0*BW0 - g1*BW1)
    out = mean(gelu(-rho))             (exact erf-based gelu)

Device dataflow (v6):
    DVE (all bf16 2x tensor_tensor):
        P1=g0*x  P2=g1*y  P3=g0*y  P4=g1*x  u=P1+P2  XU=x*u  YU=y*u
        rho = dot1m * NEG
    TensorEngine (fp16 diagonal lhsT, fp32 PSUM accumulation):
        NEG  = O0*YU - O1*XU + O2*P3 - O2*P4 - O1*g0 + O0*g1 - n0 - n1
             ( = g.BW - n0 - n1 = -r2 )
        dot1m= -V2*u + V0*g0 + V1*g1  ( = -dot1 )
        so dot1m*NEG = dot1*r2 = rho
    ACT: PSUM->bf16 copies of NEG/dot1m; gelu(-rho) with accumulate.
    GPSIMD: idle (its SBUF traffic slows concurrent DVE ops ~3.4x).

Sharding: pure data parallel, 2 batches per core, batch pair interleaved on
partition halves (batch b -> partitions [64b, 64b+64)). All pose-dependent
coefficients enter via host-built fp16 diagonal matrices whose diagonal is
per-partition (so one pass covers both batches). grad_dirs/normal_flow are
host-cast to bf16 (the device pipeline computes in bf16 anyway) and
host-interleaved to [128, 2, 4800] so each chunk is one 128-partition DMA
per tensor. x/y grids stream as fp16 (exact for ints < 2048).
Reduction: ACT accum_out -> [128, NCHUNK] partials, host sums in float64.
"""

import numpy as np
import ml_dtypes

import concourse.bacc as bacc
import concourse.bass as bass
import concourse.tile as tile
from concourse import mybir
from concourse.bass_utils import run_bass_kernel_spmd

# Problem geometry (hardcoded per the task contract).
B, H, W = 16, 480, 640
NPIX = H * W            # 307200
NCORES = 8
BPC = B // NCORES       # 2 batches per core
PHALF = 64              # partitions per batch
FTOT = NPIX // PHALF    # 4800 free elems per partition
# tapered chunks: small first chunk starts compute sooner, small last chunk
# shortens the final dependency chain
CHUNKS = [480, 960, 960, 960, 960, 480]
NCHUNK = len(CHUNKS)
FCMAX = max(CHUNKS)
MMF = 512               # max matmul free dim (one PSUM bank)

F32 = mybir.dt.float32
F16 = mybir.dt.float16
BF16 = mybir.dt.bfloat16
AF = mybir.ActivationFunctionType

# diag slots in the `diags` input
D_W0, D_W1N, D_W2, D_W2N, D_NI, D_V2N, D_V0, D_V1 = range(8)
NDIAG = 8


def _build_kernel(tc, gd, nf, xyg, diags, out):
    nc = tc.nc
    gd_t = gd.ap()
    nf_t = nf.ap()
    xy_t = xyg.ap()

    with (
        tc.tile_pool(name="singles", bufs=1) as singles,
        tc.tile_pool(name="ins", bufs=4) as ins,
        tc.tile_pool(name="mids", bufs=2) as mids,
        tc.tile_pool(name="psum", bufs=2, space="PSUM") as psp,
    ):
        dg = singles.tile([128, NDIAG, 128], F16, name="dg")
        acc = singles.tile([128, NCHUNK], F32, name="acc")

        f0s = [sum(CHUNKS[:i]) for i in range(NCHUNK)]
        for ci in range(NCHUNK):
            FC = CHUNKS[ci]
            sl = slice(f0s[ci], f0s[ci] + FC)
            gdt = ins.tile([128, 2, FCMAX], BF16, tag="gdt", name=f"gdt_{ci}")[:, :, :FC]
            nft = ins.tile([128, 2, FCMAX], BF16, tag="nft", name=f"nft_{ci}")[:, :, :FC]
            xyt = ins.tile([128, 2, FCMAX], F16, tag="xy", name=f"xy_{ci}")[:, :, :FC]
            nc.sync.dma_start(out=gdt, in_=gd_t[:, :, sl])
            nc.sync.dma_start(out=xyt, in_=xy_t[:, :, sl])
            nc.sync.dma_start(out=nft, in_=nf_t[:, :, sl])
            if ci == 0:
                # diags aren't needed until the first coefficient matmuls;
                # issue their DMA after chunk 0's data
                nc.sync.dma_start(out=dg, in_=diags.ap().rearrange("d k m -> k d m"))
                DG = [dg[:, i, :] for i in range(NDIAG)]
            g0 = gdt[:, 0]
            g1 = gdt[:, 1]
            n0 = nft[:, 0]
            n1 = nft[:, 1]
            xt = xyt[:, 0]
            yt = xyt[:, 1]

            def mtile(tag, dt=BF16):
                return mids.tile([128, FCMAX], dt, tag=tag, name=f"{tag}_{ci}")[:, :FC]

            # DVE products (all 16-bit, 2x mode)
            P1 = mtile("P1")
            nc.vector.tensor_mul(out=P1, in0=g0, in1=xt)
            P2 = mtile("P2")
            nc.vector.tensor_mul(out=P2, in0=g1, in1=yt)
            P3 = mtile("P3")
            nc.vector.tensor_mul(out=P3, in0=g0, in1=yt)
            P4 = mtile("P4")
            nc.vector.tensor_mul(out=P4, in0=g1, in1=xt)
            u = mtile("u")
            nc.vector.tensor_add(out=u, in0=P1, in1=P2)
            XU = mtile("XU")
            nc.vector.tensor_mul(out=XU, in0=xt, in1=u)
            YU = mtile("YU")
            nc.vector.tensor_mul(out=YU, in0=yt, in1=u)

            # PE: NEG = g.BW - n0 - n1 (ordered to reuse stationary weights)
            neg_ps = psp.tile([128, FCMAX], F32, tag="neg", name=f"neg_{ci}")[:, :FC]
            neg_terms = [
                (D_NI, n0), (D_NI, n1),
                (D_W0, YU), (D_W0, g1),
                (D_W1N, XU), (D_W1N, g0),
                (D_W2, P3), (D_W2N, P4),
            ]
            # PE: dot1m = -V2*u + V0*g0 + V1*g1
            d1_ps = psp.tile([128, FCMAX], F32, tag="d1", name=f"d1_{ci}")[:, :FC]
            d1_terms = [(D_V2N, u), (D_V0, g0), (D_V1, g1)]
            for ps, terms in ((neg_ps, neg_terms), (d1_ps, d1_terms)):
                for f0 in range(0, FC, MMF):
                    fs = slice(f0, min(f0 + MMF, FC))
                    for i, (di, rhs) in enumerate(terms):
                        nc.tensor.matmul(
                            ps[:, fs], DG[di], rhs[:, fs],
                            start=(i == 0), stop=(i == len(terms) - 1),
                        )

            negb = mtile("negb")
            nc.scalar.activation(out=negb, in_=neg_ps, func=AF.Copy)
            d1b = mtile("d1b")
            nc.scalar.activation(out=d1b, in_=d1_ps, func=AF.Copy)

            rho = mtile("rho")
            nc.vector.tensor_mul(out=rho, in0=d1b, in1=negb)
            gl = mtile("gl")
            nc.scalar.activation(
                out=gl, in_=rho, func=AF.Gelu, bias=0.0, scale=-1.0,
                accum_out=acc[:, ci : ci + 1],
            )

        nc.sync.dma_start(out=out.ap(), in_=acc)


def build_bass():
    nc = bacc.Bacc("TRN2", target_bir_lowering=False, debug=False)
    gd = nc.dram_tensor("gd", [128, 2, FTOT], BF16, kind="ExternalInput")
    nf = nc.dram_tensor("nf", [128, 2, FTOT], BF16, kind="ExternalInput")
    xyg = nc.dram_tensor("xyg", [128, 2, FTOT], F16, kind="ExternalInput")
    diags = nc.dram_tensor("diags", [NDIAG, 128, 128], F16, kind="ExternalInput")
    out = nc.dram_tensor("acc_out", [128, NCHUNK], F32, kind="ExternalOutput")
    with tile.TileContext(nc) as tc:
        _build_kernel(tc, gd, nf, xyg, diags, out)
    nc.compile()
    return nc


def make_in_maps(pose, grad_dirs, normal_flow):
    pose = np.asarray(pose, np.float32)
    gd = np.ascontiguousarray(np.asarray(grad_dirs, np.float32)).reshape(B, 2, NPIX)
    nf = np.ascontiguousarray(np.asarray(normal_flow, np.float32)).reshape(B, 2, NPIX)

    flat = np.arange(NPIX, dtype=np.int64).reshape(PHALF, FTOT)
    xy_half = np.stack([(flat % W), (flat // W)], axis=1)  # [64, 2, FTOT]
    xyg = np.ascontiguousarray(
        np.concatenate([xy_half, xy_half], axis=0).astype(np.float16)
    )  # [128, 2, FTOT]

    def interleave(a):
        # [BPC, 2, NPIX] -> [128, 2, FTOT] bf16; batch b -> partitions [64b, ...)
        return np.ascontiguousarray(
            a.reshape(BPC, 2, PHALF, FTOT)
            .transpose(0, 2, 1, 3)
            .reshape(128, 2, FTOT)
            .astype(ml_dtypes.bfloat16)
        )

    in_maps = []
    for core in range(NCORES):
        b0 = core * BPC
        # per-partition coefficient vectors (batch = partition // 64)
        coef = np.zeros((NDIAG, 128), np.float32)
        for h in range(BPC):
            V = pose[b0 + h, :3]
            O = pose[b0 + h, 3:]
            rows = slice(h * PHALF, (h + 1) * PHALF)
            coef[D_W0, rows] = O[0]
            coef[D_W1N, rows] = -O[1]
            coef[D_W2, rows] = O[2]
            coef[D_W2N, rows] = -O[2]
            coef[D_NI, rows] = -1.0
            coef[D_V2N, rows] = -V[2]
            coef[D_V0, rows] = V[0]
            coef[D_V1, rows] = V[1]
        diags = np.zeros((NDIAG, 128, 128), np.float16)
        for i in range(NDIAG):
            np.fill_diagonal(diags[i], coef[i].astype(np.float16))
        in_maps.append(
            {
                "gd": interleave(gd[b0 : b0 + BPC]),
                "nf": interleave(nf[b0 : b0 + BPC]),
                "xyg": xyg,
                "diags": diags,
            }
        )
    return in_maps


_NC_CACHE = None


def _get_nc():
    global _NC_CACHE
    if _NC_CACHE is None:
        _NC_CACHE = build_bass()
    return _NC_CACHE


def kernel(pose, grad_dirs, normal_flow):
    nc = _get_nc()
    in_maps = make_in_maps(pose, grad_dirs, normal_flow)
    res = run_bass_kernel_spmd(nc, in_maps, core_ids=list(range(NCORES)))
    total = 0.0
    for r in res.results:
        total += r["acc_out"].astype(np.float64).sum()
    return np.float32(total / (B * H * W))


# revision 27
# speedup vs baseline: 1.0287x; 1.0259x over previous
"""Trainium2 Bass kernel for the Cheirality loss layer.

Math (per batch b, pixel (y, x); g = grad_dirs, n = normal_flow):
    AV0 = V2*x - V0                    AV1 = V2*y - V1
    BW0 = O0*x*y - O1*(x^2+1) + O2*y   BW1 = O0*(y^2+1) - O1*x*y - O2*x
    rho = (g0*AV0 + g1*AV1) * (n0 + n1 - g0*BW0 - g1*BW1)
    out = mean(gelu(-rho))             (exact erf-based gelu)

Device dataflow (v6):
    DVE (all bf16 2x tensor_tensor):
        P1=g0*x  P2=g1*y  P3=g0*y  P4=g1*x  u=P1+P2  XU=x*u  YU=y*u
        rho = dot1m * NEG
    TensorEngine (fp16 diagonal lhsT, fp32 PSUM accumulation):
        NEG  = O0*YU - O1*XU + O2*P3 - O2*P4 - O1*g0 + O0*g1 - n0 - n1
             ( = g.BW - n0 - n1 = -r2 )
        dot1m= -V2*u + V0*g0 + V1*g1  ( = -dot1 )
        so dot1m*NEG = dot1*r2 = rho
    ACT: PSUM->bf16 copies of NEG/dot1m; gelu(-rho) with accumulate.
    GPSIMD: idle (its SBUF traffic slows concurrent DVE ops ~3.4x).

Sharding: pure data parallel, 2 batches per core, batch pair interleaved on
partition halves (batch b -> partitions [64b, 64b+64)). All pose-dependent
coefficients enter via host-built fp16 diagonal matrices whose diagonal is
per-partition (so one pass covers both batches). grad_dirs/normal_flow are
host-cast to bf16 (the device pipeline computes in bf16 anyway) and
host-interleaved to [128, 2, 4800] so each chunk is one 128-partition DMA
per tensor. x/y grids stream as fp16 (exact for ints < 2048).
Reduction: ACT accum_out -> [128, NCHUNK] partials, host sums in float64.
"""

import numpy as np
import ml_dtypes

import concourse.bacc as bacc
import concourse.bass as bass
import concourse.tile as tile
from concourse import mybir
from concourse.bass_utils import run_bass_kernel_spmd

# Problem geometry (hardcoded per the task contract).
B, H, W = 16, 480, 640
NPIX = H * W            # 307200
NCORES = 8
BPC = B // NCORES       # 2 batches per core
PHALF = 64              # partitions per batch
FTOT = NPIX // PHALF    # 4800 free elems per partition
# tapered chunks: small first chunk starts compute sooner, small last chunk
# shortens the final dependency chain
CHUNKS = [480, 960, 960, 960, 960, 480]
NCHUNK = len(CHUNKS)
FCMAX = max(CHUNKS)
MMF = 512               # max matmul free dim (one PSUM bank)

F32 = mybir.dt.float32
F16 = mybir.dt.float16
BF16 = mybir.dt.bfloat16
AF = mybir.ActivationFunctionType

# diag slots in the `diags` input
D_W0, D_W1N, D_W2, D_W2N, D_NI, D_V2N, D_V0, D_V1 = range(8)
NDIAG = 8


def _build_kernel(tc, gd, nf, xyg, diags, out):
    nc = tc.nc
    gd_t = gd.ap()
    nf_t = nf.ap()
    xy_t = xyg.ap()

    with (
        tc.tile_pool(name="singles", bufs=1) as singles,
        tc.tile_pool(name="ins", bufs=4) as ins,
        tc.tile_pool(name="mids", bufs=2) as mids,
        tc.tile_pool(name="psum", bufs=2, space="PSUM") as psp,
    ):
        dg = singles.tile([128, NDIAG, 128], F16, name="dg")
        acc = singles.tile([128, NCHUNK], F32, name="acc")

        f0s = [sum(CHUNKS[:i]) for i in range(NCHUNK)]
        for ci in range(NCHUNK):
            FC = CHUNKS[ci]
            sl = slice(f0s[ci], f0s[ci] + FC)
            gdt = ins.tile([128, 2, FCMAX], BF16, tag="gdt", name=f"gdt_{ci}")[:, :, :FC]
            nft = ins.tile([128, 2, FCMAX], BF16, tag="nft", name=f"nft_{ci}")[:, :, :FC]
            xyt = ins.tile([128, 2, FCMAX], F16, tag="xy", name=f"xy_{ci}")[:, :, :FC]
            nc.sync.dma_start(out=gdt, in_=gd_t[:, :, sl])
            nc.sync.dma_start(out=xyt, in_=xy_t[:, :, sl])
            if ci == 0:
                nc.sync.dma_start(out=dg, in_=diags.ap().rearrange("d k m -> k d m"))
                DG = [dg[:, i, :] for i in range(NDIAG)]
            nc.sync.dma_start(out=nft, in_=nf_t[:, :, sl])
            g0 = gdt[:, 0]
            g1 = gdt[:, 1]
            n0 = nft[:, 0]
            n1 = nft[:, 1]
            xt = xyt[:, 0]
            yt = xyt[:, 1]

            def mtile(tag, dt=BF16):
                return mids.tile([128, FCMAX], dt, tag=tag, name=f"{tag}_{ci}")[:, :FC]

            # DVE products (all 16-bit, 2x mode)
            P1 = mtile("P1")
            nc.vector.tensor_mul(out=P1, in0=g0, in1=xt)
            P2 = mtile("P2")
            nc.vector.tensor_mul(out=P2, in0=g1, in1=yt)
            P3 = mtile("P3")
            nc.vector.tensor_mul(out=P3, in0=g0, in1=yt)
            P4 = mtile("P4")
            nc.vector.tensor_mul(out=P4, in0=g1, in1=xt)
            u = mtile("u")
            nc.vector.tensor_add(out=u, in0=P1, in1=P2)
            XU = mtile("XU")
            nc.vector.tensor_mul(out=XU, in0=xt, in1=u)
            YU = mtile("YU")
            nc.vector.tensor_mul(out=YU, in0=yt, in1=u)

            # PE: NEG = g.BW - n0 - n1 (ordered to reuse stationary weights)
            neg_ps = psp.tile([128, FCMAX], F32, tag="neg", name=f"neg_{ci}")[:, :FC]
            neg_terms = [
                (D_NI, n0), (D_NI, n1),
                (D_W0, YU), (D_W0, g1),
                (D_W1N, XU), (D_W1N, g0),
                (D_W2, P3), (D_W2N, P4),
            ]
            # PE: dot1m = -V2*u + V0*g0 + V1*g1
            d1_ps = psp.tile([128, FCMAX], F32, tag="d1", name=f"d1_{ci}")[:, :FC]
            d1_terms = [(D_V2N, u), (D_V0, g0), (D_V1, g1)]
            for ps, terms in ((neg_ps, neg_terms), (d1_ps, d1_terms)):
                for i, (di, rhs) in enumerate(terms):
                    for f0 in range(0, FC, MMF):
                        fs = slice(f0, min(f0 + MMF, FC))
                        nc.tensor.matmul(
                            ps[:, fs], DG[di], rhs[:, fs],
                            start=(i == 0), stop=(i == len(terms) - 1),
                        )

            negb = mtile("negb")
            nc.scalar.activation(out=negb, in_=neg_ps, func=AF.Copy)
            d1b = mtile("d1b")
            nc.scalar.activation(out=d1b, in_=d1_ps, func=AF.Copy)

            rho = mtile("rho")
            nc.vector.tensor_mul(out=rho, in0=d1b, in1=negb)
            gl = mtile("gl")
            nc.scalar.activation(
                out=gl, in_=rho, func=AF.Gelu, bias=0.0, scale=-1.0,
                accum_out=acc[:, ci : ci + 1],
            )

        nc.sync.dma_start(out=out.ap(), in_=acc)


def build_bass():
    nc = bacc.Bacc("TRN2", target_bir_lowering=False, debug=False)
    gd = nc.dram_tensor("gd", [128, 2, FTOT], BF16, kind="ExternalInput")
    nf = nc.dram_tensor("nf", [128, 2, FTOT], BF16, kind="ExternalInput")
    xyg = nc.dram_tensor("xyg", [128, 2, FTOT], F16, kind="ExternalInput")
    diags = nc.dram_tensor("diags", [NDIAG, 128, 128], F16, kind="ExternalInput")
    out = nc.dram_tensor("acc_out", [128, NCHUNK], F32, kind="ExternalOutput")
    with tile.TileContext(nc) as tc:
        _build_kernel(tc, gd, nf, xyg, diags, out)
    nc.compile()
    return nc


def make_in_maps(pose, grad_dirs, normal_flow):
    pose = np.asarray(pose, np.float32)
    gd = np.ascontiguousarray(np.asarray(grad_dirs, np.float32)).reshape(B, 2, NPIX)
    nf = np.ascontiguousarray(np.asarray(normal_flow, np.float32)).reshape(B, 2, NPIX)

    flat = np.arange(NPIX, dtype=np.int64).reshape(PHALF, FTOT)
    xy_half = np.stack([(flat % W), (flat // W)], axis=1)  # [64, 2, FTOT]
    xyg = np.ascontiguousarray(
        np.concatenate([xy_half, xy_half], axis=0).astype(np.float16)
    )  # [128, 2, FTOT]

    def interleave(a):
        # [BPC, 2, NPIX] -> [128, 2, FTOT] bf16; batch b -> partitions [64b, ...)
        return np.ascontiguousarray(
            a.reshape(BPC, 2, PHALF, FTOT)
            .transpose(0, 2, 1, 3)
            .reshape(128, 2, FTOT)
            .astype(ml_dtypes.bfloat16)
        )

    in_maps = []
    for core in range(NCORES):
        b0 = core * BPC
        # per-partition coefficient vectors (batch = partition // 64)
        coef = np.zeros((NDIAG, 128), np.float32)
        for h in range(BPC):
            V = pose[b0 + h, :3]
            O = pose[b0 + h, 3:]
            rows = slice(h * PHALF, (h + 1) * PHALF)
            coef[D_W0, rows] = O[0]
            coef[D_W1N, rows] = -O[1]
            coef[D_W2, rows] = O[2]
            coef[D_W2N, rows] = -O[2]
            coef[D_NI, rows] = -1.0
            coef[D_V2N, rows] = -V[2]
            coef[D_V0, rows] = V[0]
            coef[D_V1, rows] = V[1]
        diags = np.zeros((NDIAG, 128, 128), np.float16)
        for i in range(NDIAG):
            np.fill_diagonal(diags[i], coef[i].astype(np.float16))
        in_maps.append(
            {
                "gd": interleave(gd[b0 : b0 + BPC]),
                "nf": interleave(nf[b0 : b0 + BPC]),
                "xyg": xyg,
                "diags": diags,
            }
        )
    return in_maps


_NC_CACHE = None


def _get_nc():
    global _NC_CACHE
    if _NC_CACHE is None:
        _NC_CACHE = build_bass()
    return _NC_CACHE


def kernel(pose, grad_dirs, normal_flow):
    nc = _get_nc()
    in_maps = make_in_maps(pose, grad_dirs, normal_flow)
    res = run_bass_kernel_spmd(nc, in_maps, core_ids=list(range(NCORES)))
    total = 0.0
    for r in res.results:
        total += r["acc_out"].astype(np.float64).sum()
    return np.float32(total / (B * H * W))


# revision 28
# speedup vs baseline: 1.0595x; 1.0299x over previous
"""Trainium2 Bass kernel for the Cheirality loss layer.

Math (per batch b, pixel (y, x); g = grad_dirs, n = normal_flow):
    AV0 = V2*x - V0                    AV1 = V2*y - V1
    BW0 = O0*x*y - O1*(x^2+1) + O2*y   BW1 = O0*(y^2+1) - O1*x*y - O2*x
    rho = (g0*AV0 + g1*AV1) * (n0 + n1 - g0*BW0 - g1*BW1)
    out = mean(gelu(-rho))             (exact erf-based gelu)

Device dataflow (v6):
    DVE (all bf16 2x tensor_tensor):
        P1=g0*x  P2=g1*y  P3=g0*y  P4=g1*x  u=P1+P2  XU=x*u  YU=y*u
        rho = dot1m * NEG
    TensorEngine (fp16 diagonal lhsT, fp32 PSUM accumulation):
        NEG  = O0*YU - O1*XU + O2*P3 - O2*P4 - O1*g0 + O0*g1 - n0 - n1
             ( = g.BW - n0 - n1 = -r2 )
        dot1m= -V2*u + V0*g0 + V1*g1  ( = -dot1 )
        so dot1m*NEG = dot1*r2 = rho
    ACT: PSUM->bf16 copies of NEG/dot1m; gelu(-rho) with accumulate.
    GPSIMD: idle (its SBUF traffic slows concurrent DVE ops ~3.4x).

Sharding: pure data parallel, 2 batches per core, batch pair interleaved on
partition halves (batch b -> partitions [64b, 64b+64)). All pose-dependent
coefficients enter via host-built fp16 diagonal matrices whose diagonal is
per-partition (so one pass covers both batches). grad_dirs/normal_flow are
host-cast to bf16 (the device pipeline computes in bf16 anyway) and
host-interleaved to [128, 2, 4800] so each chunk is one 128-partition DMA
per tensor. x/y grids stream as fp16 (exact for ints < 2048).
Reduction: ACT accum_out -> [128, NCHUNK] partials, host sums in float64.
"""

import numpy as np
import ml_dtypes

import concourse.bacc as bacc
import concourse.bass as bass
import concourse.tile as tile
from concourse import mybir
from concourse.bass_utils import run_bass_kernel_spmd

# Problem geometry (hardcoded per the task contract).
B, H, W = 16, 480, 640
NPIX = H * W            # 307200
NCORES = 8
BPC = B // NCORES       # 2 batches per core
PHALF = 64              # partitions per batch
FTOT = NPIX // PHALF    # 4800 free elems per partition
# tapered chunks: small first chunk starts compute sooner, small last chunk
# shortens the final dependency chain
CHUNKS = [480, 960, 960, 960, 960, 480]
NCHUNK = len(CHUNKS)
FCMAX = max(CHUNKS)
MMF = 512               # max matmul free dim (one PSUM bank)

F32 = mybir.dt.float32
F16 = mybir.dt.float16
BF16 = mybir.dt.bfloat16
AF = mybir.ActivationFunctionType

# diag slots in the `diags` input
D_W0, D_W1N, D_W2, D_W2N, D_NI, D_V2N, D_V0, D_V1 = range(8)
NDIAG = 8


def _build_kernel(tc, gd, nf, xyg, diags, out):
    nc = tc.nc
    gd_t = gd.ap()
    nf_t = nf.ap()
    xy_t = xyg.ap()

    with (
        tc.tile_pool(name="singles", bufs=1) as singles,
        tc.tile_pool(name="ins", bufs=4) as ins,
        tc.tile_pool(name="mids", bufs=2) as mids,
        tc.tile_pool(name="psum", bufs=2, space="PSUM") as psp,
    ):
        dg = singles.tile([128, NDIAG, 128], F16, name="dg")
        acc = singles.tile([128, NCHUNK], F32, name="acc")

        f0s = [sum(CHUNKS[:i]) for i in range(NCHUNK)]
        for ci in range(NCHUNK):
            FC = CHUNKS[ci]
            sl = slice(f0s[ci], f0s[ci] + FC)
            gdt = ins.tile([128, 2, FCMAX], BF16, tag="gdt", name=f"gdt_{ci}")[:, :, :FC]
            nft = ins.tile([128, 2, FCMAX], BF16, tag="nft", name=f"nft_{ci}")[:, :, :FC]
            xyt = ins.tile([128, 2, FCMAX], F16, tag="xy", name=f"xy_{ci}")[:, :, :FC]
            nc.sync.dma_start(out=gdt, in_=gd_t[:, :, sl])
            nc.sync.dma_start(out=xyt, in_=xy_t[:, :, sl])
            if ci == 0:
                nc.sync.dma_start(out=dg, in_=diags.ap().rearrange("d k m -> k d m"))
                DG = [dg[:, i, :] for i in range(NDIAG)]
            nc.sync.dma_start(out=nft, in_=nf_t[:, :, sl])
            g0 = gdt[:, 0]
            g1 = gdt[:, 1]
            n0 = nft[:, 0]
            n1 = nft[:, 1]
            xt = xyt[:, 0]
            yt = xyt[:, 1]

            def mtile(tag, dt=BF16):
                return mids.tile([128, FCMAX], dt, tag=tag, name=f"{tag}_{ci}")[:, :FC]

            # DVE products (all 16-bit, 2x mode)
            P1 = mtile("P1")
            nc.vector.tensor_mul(out=P1, in0=g0, in1=xt)
            P2 = mtile("P2")
            nc.vector.tensor_mul(out=P2, in0=g1, in1=yt)
            u = mtile("u")
            nc.vector.tensor_add(out=u, in0=P1, in1=P2)
            P3 = mtile("P3")
            nc.vector.tensor_mul(out=P3, in0=g0, in1=yt)
            P4 = mtile("P4")
            nc.vector.tensor_mul(out=P4, in0=g1, in1=xt)
            XU = mtile("XU")
            nc.vector.tensor_mul(out=XU, in0=xt, in1=u)
            YU = mtile("YU")
            nc.vector.tensor_mul(out=YU, in0=yt, in1=u)

            # PE: NEG = g.BW - n0 - n1 (ordered to reuse stationary weights)
            neg_ps = psp.tile([128, FCMAX], F32, tag="neg", name=f"neg_{ci}")[:, :FC]
            neg_terms = [
                (D_NI, n0), (D_NI, n1),
                (D_W0, g1), (D_W1N, g0),
                (D_W2, P3), (D_W2N, P4),
                (D_W0, YU), (D_W1N, XU),
            ]
            # PE: dot1m = -V2*u + V0*g0 + V1*g1
            d1_ps = psp.tile([128, FCMAX], F32, tag="d1", name=f"d1_{ci}")[:, :FC]
            d1_terms = [(D_V0, g0), (D_V1, g1), (D_V2N, u)]
            for ps, terms in ((d1_ps, d1_terms), (neg_ps, neg_terms)):
                for i, (di, rhs) in enumerate(terms):
                    for f0 in range(0, FC, MMF):
                        fs = slice(f0, min(f0 + MMF, FC))
                        nc.tensor.matmul(
                            ps[:, fs], DG[di], rhs[:, fs],
                            start=(i == 0), stop=(i == len(terms) - 1),
                        )

            negb = mtile("negb")
            nc.scalar.activation(out=negb, in_=neg_ps, func=AF.Copy)
            d1b = mtile("d1b")
            nc.scalar.activation(out=d1b, in_=d1_ps, func=AF.Copy)

            rho = mtile("rho")
            nc.vector.tensor_mul(out=rho, in0=d1b, in1=negb)
            gl = mtile("gl")
            nc.scalar.activation(
                out=gl, in_=rho, func=AF.Gelu, bias=0.0, scale=-1.0,
                accum_out=acc[:, ci : ci + 1],
            )

        nc.sync.dma_start(out=out.ap(), in_=acc)


def build_bass():
    nc = bacc.Bacc("TRN2", target_bir_lowering=False, debug=False)
    gd = nc.dram_tensor("gd", [128, 2, FTOT], BF16, kind="ExternalInput")
    nf = nc.dram_tensor("nf", [128, 2, FTOT], BF16, kind="ExternalInput")
    xyg = nc.dram_tensor("xyg", [128, 2, FTOT], F16, kind="ExternalInput")
    diags = nc.dram_tensor("diags", [NDIAG, 128, 128], F16, kind="ExternalInput")
    out = nc.dram_tensor("acc_out", [128, NCHUNK], F32, kind="ExternalOutput")
    with tile.TileContext(nc) as tc:
        _build_kernel(tc, gd, nf, xyg, diags, out)
    nc.compile()
    return nc


def make_in_maps(pose, grad_dirs, normal_flow):
    pose = np.asarray(pose, np.float32)
    gd = np.ascontiguousarray(np.asarray(grad_dirs, np.float32)).reshape(B, 2, NPIX)
    nf = np.ascontiguousarray(np.asarray(normal_flow, np.float32)).reshape(B, 2, NPIX)

    flat = np.arange(NPIX, dtype=np.int64).reshape(PHALF, FTOT)
    xy_half = np.stack([(flat % W), (flat // W)], axis=1)  # [64, 2, FTOT]
    xyg = np.ascontiguousarray(
        np.concatenate([xy_half, xy_half], axis=0).astype(np.float16)
    )  # [128, 2, FTOT]

    def interleave(a):
        # [BPC, 2, NPIX] -> [128, 2, FTOT] bf16; batch b -> partitions [64b, ...)
        return np.ascontiguousarray(
            a.reshape(BPC, 2, PHALF, FTOT)
            .transpose(0, 2, 1, 3)
            .reshape(128, 2, FTOT)
            .astype(ml_dtypes.bfloat16)
        )

    in_maps = []
    for core in range(NCORES):
        b0 = core * BPC
        # per-partition coefficient vectors (batch = partition // 64)
        coef = np.zeros((NDIAG, 128), np.float32)
        for h in range(BPC):
            V = pose[b0 + h, :3]
            O = pose[b0 + h, 3:]
            rows = slice(h * PHALF, (h + 1) * PHALF)
            coef[D_W0, rows] = O[0]
            coef[D_W1N, rows] = -O[1]
            coef[D_W2, rows] = O[2]
            coef[D_W2N, rows] = -O[2]
            coef[D_NI, rows] = -1.0
            coef[D_V2N, rows] = -V[2]
            coef[D_V0, rows] = V[0]
            coef[D_V1, rows] = V[1]
        diags = np.zeros((NDIAG, 128, 128), np.float16)
        for i in range(NDIAG):
            np.fill_diagonal(diags[i], coef[i].astype(np.float16))
        in_maps.append(
            {
                "gd": interleave(gd[b0 : b0 + BPC]),
                "nf": interleave(nf[b0 : b0 + BPC]),
                "xyg": xyg,
                "diags": diags,
            }
        )
    return in_maps


_NC_CACHE = None


def _get_nc():
    global _NC_CACHE
    if _NC_CACHE is None:
        _NC_CACHE = build_bass()
    return _NC_CACHE


def kernel(pose, grad_dirs, normal_flow):
    nc = _get_nc()
    in_maps = make_in_maps(pose, grad_dirs, normal_flow)
    res = run_bass_kernel_spmd(nc, in_maps, core_ids=list(range(NCORES)))
    total = 0.0
    for r in res.results:
        total += r["acc_out"].astype(np.float64).sum()
    return np.float32(total / (B * H * W))
